# revision 1
# baseline (speedup 1.0000x reference)
"""GeomGCN (2-layer relational GCN) distributed Bass kernel for 8 TRN2 NeuronCores.

Strategy (source-sharded, graph-parallel):
  - Nodes are split into 8 contiguous slices of NLOC; core k owns slice k.
  - Math: with dinv = deg^-1/2 (deg = in-degree by `row` over all edges),
      h1 = sum_r A_r (dinv*x) @ W1_r  scaled by dinv on the dest side, + b1
    Each core computes y1 = (dinv*x_k) @ W1_r for its node slice (dense
    matmuls), stores a per-relation message table in DRAM, then gathers
    per-edge rows with dma_gather (edges assigned to the core that owns the
    edge's source node `col`).  Edges are host-sorted by destination and
    padded into 128-edge chunks per 128-dest tile; a one-hot selection
    matrix (built on-device via iota/is_equal) turns the per-tile
    segment-sum into TensorE matmuls accumulated in PSUM.  Per-node partial
    sums [N_PAD, H] are combined with a ReduceScatter so each core ends up
    with its own node slice of h1; layer 2 repeats the same pattern with
    16-wide messages, then log_softmax.
  Host work is index-only: slicing, sorting, degree counts, chunk/slot
  assignment and int16 gather-index tables.
"""
import math
import os
import numpy as np

import concourse.bass as bass
import concourse.tile as tile
from concourse import bacc, mybir
from concourse.bass_utils import run_bass_kernel_spmd

F32 = mybir.dt.float32
I16 = mybir.dt.int16
AF = mybir.ActivationFunctionType
ALU = mybir.AluOpType


class Cfg:
    def __init__(self, N, E, F, H, C, R, ncores=8, B=64, J=8):
        self.N, self.E, self.F, self.H, self.C, self.R = N, E, F, H, C, R
        self.ncores = ncores
        self.P = 128
        # node slice per core, padded so N_PAD = ncores * NLOC and NLOC
        # covers the largest slice
        self.NLOC = math.ceil(math.ceil(N / ncores) / 16) * 16
        while (self.NLOC * ncores) % 128 != 0:
            self.NLOC += 16
        self.N_PAD = self.NLOC * ncores
        self.NT = self.N_PAD // 128                  # dest tiles
        self.MC = math.ceil(self.NLOC / 128)         # m-chunks per slice
        self.MPAD = self.MC * 128
        self.YSTRIDE = self.MPAD + 128               # rows per relation in table
        self.DUMMY = self.MPAD                       # a guaranteed-zero table row
        self.E1 = H                                  # layer-1 message width
        self.E2 = max(64, C)                         # layer-2 row (256B min)
        self.B = B                                   # gather chunks per batch
        self.J = J                                   # S^T chunks per build
        self.KC = F // 128                           # k-chunks layer-1 dense
        assert F % 128 == 0 and H == 128 and self.YSTRIDE % 16 == 0
        assert self.R * self.YSTRIDE < 32768, "int16 gather index overflow"


CFG = Cfg(N=50000, E=800000, F=256, H=128, C=16, R=4, B=8)


# ----------------------------------------------------------------- host side
def preprocess(cfg, x, edge_index, edge_relation, W1, b1, W2, b2):
    N, ncores, NLOC, NT = cfg.N, cfg.ncores, cfg.NLOC, cfg.NT
    row = np.asarray(edge_index[0], dtype=np.int64)
    col = np.asarray(edge_index[1], dtype=np.int64)
    rel = np.asarray(edge_relation, dtype=np.int64)
    x = np.asarray(x, dtype=np.float32)

    deg = np.bincount(row, minlength=N).astype(np.float32)

    # per-core edge sets (by source/col ownership), sorted by dest row
    per_core = []
    counts = np.zeros((ncores, NT), dtype=np.int64)
    for k in range(ncores):
        m = (col // NLOC) == k
        er, ec, eg = row[m], col[m] - k * NLOC, rel[m]
        o = np.argsort(er, kind="stable")
        er, ec, eg = er[o], ec[o], eg[o]
        t = er // 128
        counts[k] = np.bincount(t, minlength=NT)
        per_core.append((er, ec, eg, t))

    # chunks per dest tile = max over cores (>=1), shared static schedule
    chunks_t = np.maximum(1, np.ceil(counts.max(axis=0) / 128).astype(np.int64))
    CH = int(chunks_t.sum())
    CHpad = math.ceil(CH / cfg.B) * cfg.B
    NB = CHpad // cfg.B
    slot_base = np.concatenate([[0], np.cumsum(chunks_t * 128)])[:-1]

    in_maps = []
    iota = np.broadcast_to(np.arange(128, dtype=np.float32), (128, 128)).copy()
    ident = np.eye(128, dtype=np.float32)
    for k in range(ncores):
        er, ec, eg, t = per_core[k]
        first = np.searchsorted(t, np.arange(NT), side="left")
        rank = np.arange(len(t)) - first[t]
        slots = slot_base[t] + rank
        gidx = np.full(CHpad * 128, cfg.DUMMY, dtype=np.int16)
        gidx[slots] = (eg * cfg.YSTRIDE + ec).astype(np.int16)
        dloc = np.zeros(CHpad * 128, dtype=np.float32)
        dloc[slots] = (er % 128).astype(np.float32)

        # wrapped-16 int16 index layout per batch, replicated to 8 groups
        g = gidx.reshape(NB, cfg.B * 8, 16)              # [b, s, r]
        w = np.transpose(g, (0, 2, 1))                   # [b, r, s]
        gidx_w = np.broadcast_to(
            w[:, None, :, :], (NB, 8, 16, cfg.B * 8)
        ).reshape(NB * 128, cfg.B * 8).astype(np.int16)
        dloc_w = np.ascontiguousarray(
            dloc.reshape(CHpad, 128).T
        ).reshape(128, CHpad, 1)

        lo = k * NLOC
        hi = min(N, lo + NLOC)
        xk = np.zeros((cfg.MPAD, cfg.F), dtype=np.float32)
        xk[: hi - lo] = x[lo:hi]
        dk = np.zeros(cfg.MPAD, dtype=np.float32)
        dk[: hi - lo] = deg[lo:hi]

        in_maps.append({
            "xT": np.ascontiguousarray(xk.T),
            "degc": np.ascontiguousarray(dk.reshape(cfg.MC, 128).T),
            "W1": np.asarray(W1, dtype=np.float32),
            "W2": np.asarray(W2, dtype=np.float32),
            "b1c": np.asarray(b1, dtype=np.float32).reshape(cfg.H, 1),
            "b2r": np.broadcast_to(np.asarray(b2, dtype=np.float32),
                                   (128, cfg.C)).copy(),
            "iota": iota.reshape(128, 1, 128),
            "ident": ident,
            "gidx": gidx_w,
            "dloc": dloc_w,
        })
    return in_maps, tuple(int(v) for v in chunks_t), CHpad


# --------------------------------------------------------------- device side
def build_program(cfg, chunks_t, CHpad):
    P, R, H, C = cfg.P, cfg.R, cfg.H, cfg.C
    NB = CHpad // cfg.B
    nc = bacc.Bacc("TRN2", target_bir_lowering=False, debug=False,
                   num_devices=cfg.ncores)

    xT = nc.dram_tensor("xT", [cfg.F, cfg.MPAD], F32, kind="ExternalInput").ap()
    degc = nc.dram_tensor("degc", [128, cfg.MC], F32, kind="ExternalInput").ap()
    W1 = nc.dram_tensor("W1", [R * cfg.F, H], F32, kind="ExternalInput").ap()
    W2 = nc.dram_tensor("W2", [R * H, C], F32, kind="ExternalInput").ap()
    b1c = nc.dram_tensor("b1c", [H, 1], F32, kind="ExternalInput").ap()
    b2r = nc.dram_tensor("b2r", [128, C], F32, kind="ExternalInput").ap()
    iota = nc.dram_tensor("iota", [128, 1, 128], F32, kind="ExternalInput").ap()
    ident = nc.dram_tensor("ident", [128, 128], F32, kind="ExternalInput").ap()
    gidx = nc.dram_tensor("gidx", [NB * 128, cfg.B * 8], I16,
                          kind="ExternalInput").ap()
    dloc = nc.dram_tensor("dloc", [128, CHpad, 1], F32,
                          kind="ExternalInput").ap()
    out = nc.dram_tensor("out", [cfg.NLOC, C], F32, kind="ExternalOutput").ap()

    with tile.TileContext(nc) as tc:
        _build(tc, cfg, chunks_t, CHpad, xT, degc, W1, W2, b1c, b2r,
               iota, ident, gidx, dloc, out)
    nc.compile()
    return nc


def _build(tc, cfg, chunks_t, CHpad, xT, degc, W1, W2, b1c, b2r,
           iota, ident, gidx, dloc, out):
    nc = tc.nc
    P, R, H, C = cfg.P, cfg.R, cfg.H, cfg.C
    B, J, MC, NT = cfg.B, cfg.J, cfg.MC, cfg.NT
    NB = CHpad // B
    with tc.tile_pool(name="const", bufs=1) as cpool, \
         tc.tile_pool(name="big", bufs=1) as bigp, \
         tc.tile_pool(name="gY", bufs=3) as gpool, \
         tc.tile_pool(name="idx", bufs=3) as ipool, \
         tc.tile_pool(name="s3", bufs=3) as spool, \
         tc.tile_pool(name="stage", bufs=6) as stpool, \
         tc.tile_pool(name="psum", bufs=6, space="PSUM") as pp, \
         tc.tile_pool(name="dram", bufs=1, space="DRAM") as dram:

        # ---------- constants / degree scaling
        iota_t = cpool.tile([128, 1, 128], F32)
        nc.sync.dma_start(out=iota_t[:], in_=iota[:, :, :])
        ident_t = cpool.tile([128, 128], F32)
        nc.sync.dma_start(out=ident_t[:], in_=ident[:, :])
        b2_t = cpool.tile([128, C], F32)
        nc.sync.dma_start(out=b2_t[:], in_=b2r[:, :])
        b1_t = cpool.tile([H, 1], F32)
        nc.sync.dma_start(out=b1_t[:], in_=b1c[:, :])
        w1b = {}
        for r in range(R):
            for kc in range(cfg.KC):
                t = cpool.tile([128, H], F32, tag=f"w1_{r}_{kc}")
                nc.sync.dma_start(
                    out=t[:], in_=W1[r * cfg.F + kc * 128:
                                     r * cfg.F + (kc + 1) * 128, :])
                w1b[(r, kc)] = t
        w2b = {}
        for r in range(R):
            t = cpool.tile([H, C], F32, tag=f"w2_{r}")
            nc.sync.dma_start(out=t[:], in_=W2[r * H:(r + 1) * H, :])
            w2b[r] = t

        def make_dinv(src_ap, shape, tag):
            d = cpool.tile(shape, F32, tag=f"deg_{tag}")
            nc.sync.dma_start(out=d[:], in_=src_ap)
            mask = cpool.tile(shape, F32, tag=f"m_{tag}")
            nc.vector.tensor_scalar(out=mask[:], in0=d[:], scalar1=0.0,
                                    scalar2=None, op0=ALU.is_gt)
            s = cpool.tile(shape, F32, tag=f"s_{tag}")
            nc.scalar.sqrt(out=s[:], in_=d[:])
            rcp = cpool.tile(shape, F32, tag=f"r_{tag}")
            nc.vector.reciprocal(out=rcp[:], in_=s[:])
            dv = cpool.tile(shape, F32, tag=f"dv_{tag}")
            nc.vector.tensor_mul(out=dv[:], in0=rcp[:], in1=mask[:])
            return dv

        dinv_c = make_dinv(degc[:, :], [128, MC], "c")         # col layout
        dinv2_c = cpool.tile([128, MC], F32)
        nc.vector.tensor_mul(out=dinv2_c[:], in0=dinv_c[:], in1=dinv_c[:])

        # ---------- layer-1 dense: y1[r*YS + m, :] = (dinv*x)[m] @ W1_r
        uT = {}
        for kc in range(cfg.KC):
            t = bigp.tile([128, cfg.MPAD], F32, tag=f"uT{kc}")
            nc.sync.dma_start(out=t[:], in_=xT[kc * 128:(kc + 1) * 128, :])
            uT[kc] = t

        y1_dram = dram.tile([R * cfg.YSTRIDE, H], F32)
        for r in range(R):
            for mc in range(MC):
                ps = pp.tile([128, H], F32, tag="ps")
                for kc in range(cfg.KC):
                    nc.tensor.matmul(
                        out=ps[:],
                        lhsT=uT[kc][:, mc * 128:(mc + 1) * 128],
                        rhs=w1b[(r, kc)][:],
                        start=(kc == 0), stop=(kc == cfg.KC - 1))
                st = stpool.tile([128, H], F32, tag="ev1")
                nc.scalar.mul(out=st[:], in_=ps[:], mul=dinv_c[:, mc:mc + 1])
                nc.sync.dma_start(
                    out=y1_dram[r * cfg.YSTRIDE + mc * 128:
                                r * cfg.YSTRIDE + (mc + 1) * 128, :],
                    in_=st[:])
        # zero the dummy rows (row DUMMY..DUMMY+127 per relation)
        zt = cpool.tile([128, H], F32)
        nc.vector.memset(zt[:], 0.0)
        for r in range(R):
            nc.sync.dma_start(
                out=y1_dram[r * cfg.YSTRIDE + cfg.DUMMY:
                            r * cfg.YSTRIDE + cfg.DUMMY + 128, :],
                in_=zt[:])

        LIMIT = int(os.environ.get("KLIMIT", "6"))
        if LIMIT < 2:
            return
        # ---------- shared: dest-local ids for one-hot building
        dloc_t = bigp.tile([128, CHpad, 1], F32)
        nc.sync.dma_start(out=dloc_t[:], in_=dloc[:, :, :])

        def agg_pass(table_ap, elem, width, part_dram, tagsfx):
            """Gather + one-hot matmul segment sum; stream per-tile results."""
            batches = {}

            def batch(b):
                if b not in batches:
                    it = ipool.tile([128, B * 8], I16, tag="idx")
                    nc.sync.dma_start(
                        out=it[:], in_=gidx[b * 128:(b + 1) * 128, :])
                    g = gpool.tile([128, B, elem], F32, tag="g")
                    nc.gpsimd.dma_gather(
                        out_ap=g[:], in_ap=table_ap, idxs_ap=it[:],
                        num_idxs=B * 128, num_idxs_reg=B * 128,
                        elem_size=elem)
                    batches[b] = g
                return batches[b]

            s3 = None
            c = 0
            for t in range(NT):
                ps = pp.tile([128, width], F32, tag="ps")
                for j in range(chunks_t[t]):
                    g = batch(c // B)
                    if c % J == 0:
                        s3 = spool.tile([128, J, 128], F32, tag="s3")
                        nj = min(J, CHpad - c)
                        nc.vector.tensor_tensor(
                            out=s3[:, :nj, :],
                            in0=dloc_t[:, c:c + nj, :].to_broadcast(
                                [128, nj, 128]),
                            in1=iota_t[:].to_broadcast([128, nj, 128]),
                            op=ALU.is_equal)
                    nc.tensor.matmul(
                        out=ps[:], lhsT=s3[:, c % J, :],
                        rhs=g[:, c % B, :width],
                        start=(j == 0), stop=(j == chunks_t[t] - 1))
                    c += 1
                st = stpool.tile([128, width], F32, tag=f"ev{tagsfx}")
                nc.scalar.copy(out=st[:], in_=ps[:])
                nc.sync.dma_start(
                    out=part_dram[t * 128:(t + 1) * 128, :], in_=st[:])

        # ---------- layer-1 aggregation + reduce-scatter
        t1_part = dram.tile([cfg.N_PAD, H], F32)
        t1_red = dram.tile([cfg.NLOC, H], F32)
        agg_pass(y1_dram[:, :], H, H, t1_part, "1")
        if LIMIT < 3:
            return
        nc.gpsimd.collective_compute(
            "ReduceScatter", ALU.add,
            replica_groups=[list(range(cfg.ncores))],
            ins=[t1_part.opt()], outs=[t1_red.opt()])

        if LIMIT < 4:
            return
        # ---------- layer-2 dense: y2[r*YS+m, :C] = u2[m] @ W2_r + dinv[m]*c_r
        # u2 = dinv^2 * t1 + dinv * b1  (h1 = dinv*t1 + b1 folded in)
        ones_t = cpool.tile([1, 128], F32)
        nc.vector.memset(ones_t[:], 1.0)
        crow = {}
        for r in range(R):
            ps = pp.tile([1, C], F32, tag="ps")
            nc.tensor.matmul(out=ps[:], lhsT=b1_t[:], rhs=w2b[r][:],
                             start=True, stop=True)
            ct = cpool.tile([1, C], F32, tag=f"c_{r}")
            nc.scalar.copy(out=ct[:], in_=ps[:])
            psb = pp.tile([128, C], F32, tag="ps")
            nc.tensor.matmul(out=psb[:], lhsT=ones_t[:], rhs=ct[:],
                             start=True, stop=True)
            cb = cpool.tile([128, C], F32, tag=f"cb_{r}")
            nc.scalar.copy(out=cb[:], in_=psb[:])
            crow[r] = cb

        u2T = bigp.tile([128, cfg.MPAD], F32)
        nrows_last = cfg.NLOC - (MC - 1) * 128
        for mc in range(MC):
            tt = stpool.tile([128, H], F32, tag="tt")
            rows = 128 if mc < MC - 1 else nrows_last
            if rows < 128:
                nc.vector.memset(tt[:], 0.0)
            nc.sync.dma_start(out=tt[:rows, :],
                              in_=t1_red[mc * 128:mc * 128 + rows, :])
            nc.vector.tensor_scalar(out=tt[:], in0=tt[:],
                                    scalar1=dinv2_c[:, mc:mc + 1],
                                    scalar2=None, op0=ALU.mult)
            ps = pp.tile([128, 128], F32, tag="ps")
            nc.tensor.transpose(out=ps[:], in_=tt[:], identity=ident_t[:])
            nc.scalar.copy(out=u2T[:, mc * 128:(mc + 1) * 128], in_=ps[:])

        y2_dram = dram.tile([R * cfg.YSTRIDE, cfg.E2], F32)
        for r in range(R):
            for mc in range(MC):
                ps = pp.tile([128, C], F32, tag="ps")
                nc.tensor.matmul(out=ps[:],
                                 lhsT=u2T[:, mc * 128:(mc + 1) * 128],
                                 rhs=w2b[r][:], start=True, stop=True)
                st = stpool.tile([128, cfg.E2], F32, tag="ev2w")
                nc.vector.memset(st[:], 0.0)
                nc.vector.tensor_scalar(
                    out=st[:, :C], in0=crow[r][:],
                    scalar1=dinv_c[:, mc:mc + 1], scalar2=None, op0=ALU.mult)
                nc.vector.tensor_tensor(out=st[:, :C], in0=st[:, :C],
                                        in1=ps[:], op=ALU.add)
                nc.sync.dma_start(
                    out=y2_dram[r * cfg.YSTRIDE + mc * 128:
                                r * cfg.YSTRIDE + (mc + 1) * 128, :],
                    in_=st[:])
        zt2 = cpool.tile([128, cfg.E2], F32)
        nc.vector.memset(zt2[:], 0.0)
        for r in range(R):
            nc.sync.dma_start(
                out=y2_dram[r * cfg.YSTRIDE + cfg.DUMMY:
                            r * cfg.YSTRIDE + cfg.DUMMY + 128, :],
                in_=zt2[:])

        if LIMIT < 5:
            return
        # ---------- layer-2 aggregation + reduce-scatter
        t2_part = dram.tile([cfg.N_PAD, C], F32)
        t2_red = dram.tile([cfg.NLOC, C], F32)
        agg_pass(y2_dram[:, :], cfg.E2, C, t2_part, "2")
        nc.gpsimd.collective_compute(
            "ReduceScatter", ALU.add,
            replica_groups=[list(range(cfg.ncores))],
            ins=[t2_part.opt()], outs=[t2_red.opt()])

        if LIMIT < 6:
            return
        # ---------- final: h2 = dinv*t2 + b2 ; log_softmax rows
        for mc in range(MC):
            rows = 128 if mc < MC - 1 else nrows_last
            ft = stpool.tile([128, C], F32, tag="fin")
            nc.sync.dma_start(out=ft[:rows, :],
                              in_=t2_red[mc * 128:mc * 128 + rows, :])
            nc.vector.tensor_scalar(out=ft[:], in0=ft[:],
                                    scalar1=dinv_c[:, mc:mc + 1],
                                    scalar2=None, op0=ALU.mult)
            nc.vector.tensor_tensor(out=ft[:], in0=ft[:],
                                    in1=b2_t[:], op=ALU.add)
            negmx = stpool.tile([128, 1], F32, tag="mx")
            nc.vector.tensor_reduce(out=negmx[:], in_=ft[:],
                                    axis=mybir.AxisListType.X,
                                    op=ALU.max, negate=True)
            ex = stpool.tile([128, C], F32, tag="ex")
            ssum = stpool.tile([128, 1], F32, tag="sm")
            nc.scalar.activation(out=ex[:], in_=ft[:], func=AF.Exp,
                                 bias=negmx[:, 0:1], scale=1.0,
                                 accum_out=ssum[:, 0:1])
            lg = stpool.tile([128, 1], F32, tag="lg")
            nc.scalar.activation(out=lg[:], in_=ssum[:], func=AF.Ln)
            nc.vector.tensor_scalar(out=ft[:], in0=ft[:],
                                    scalar1=negmx[:, 0:1],
                                    scalar2=lg[:, 0:1],
                                    op0=ALU.add, op1=ALU.subtract)
            nc.sync.dma_start(out=out[mc * 128:mc * 128 + rows, :],
                              in_=ft[:rows, :])


# ------------------------------------------------------------------ runtime
_PROGRAM_CACHE = {}


def run(cfg, inputs):
    in_maps, chunks_t, CHpad = preprocess(cfg, **inputs)
    key = (cfg.N, cfg.E, chunks_t, CHpad)
    if key not in _PROGRAM_CACHE:
        _PROGRAM_CACHE[key] = build_program(cfg, chunks_t, CHpad)
    nc = _PROGRAM_CACHE[key]
    res = None
    for attempt in range(3):
        try:
            res = run_bass_kernel_spmd(nc, in_maps,
                                       core_ids=list(range(cfg.ncores)))
            break
        except Exception:
            if attempt == 2:
                raise
    outs = [res.results[k]["out"][:cfg.NLOC] for k in range(cfg.ncores)]
    full = np.concatenate(outs, axis=0)[:cfg.N]
    return np.ascontiguousarray(full.astype(np.float32))


def kernel(x, edge_index, edge_relation, W1, b1, W2, b2):
    return run(CFG, dict(x=x, edge_index=edge_index,
                         edge_relation=edge_relation,
                         W1=W1, b1=b1, W2=W2, b2=b2))



# revision 6
# speedup vs baseline: 1.4855x; 1.4855x over previous
"""GeomGCN (2-layer relational GCN) distributed Bass kernel for 8 TRN2 NeuronCores.

Strategy (source-sharded, graph-parallel, bf16 hot path):
  - Nodes split into 8 slices of NLOC (multiple of 128); core k owns slice k
    and all edges whose source `col` lies in it.
  - Layer tables are stored (node, rel)-combined: row index = node*4 + rel,
    so a whole node's 4 relation messages are 1KB-contiguous and each table
    is written with ONE fat DMA.  Both layers share one gather-index table
    (idx = col_local*4 + rel), resident in SBUF.
  - Layer-1 aggregation is TRANSPOSED on TensorE (lhsT = gathered messages,
    rhs = one-hot S) so partial sums come out [H, node]; after the
    ReduceScatter the reduced block is already the lhsT for layer-2 dense.
  - S one-hot matrices are built per 128-edge chunk with a single
    tensor_scalar(is_equal) against a resident iota row (DVE 4x mode, bf16).
  - Partial sums are staged in SBUF per dest core and written with 8 DMAs,
    then combined with a bf16 ReduceScatter.  Layer 2 repeats with 16-wide
    messages (node-major matmuls), then a fused log_softmax (2 activation
    table loads total) emits a transposed [128, MC*16] output that the host
    un-transposes.
  Host work: index prep, degree/dinv, pre-scaling x by dinv, bf16 packing.
"""
import math
import os
import numpy as np
import ml_dtypes

import concourse.bass as bass
import concourse.tile as tile
from concourse import bacc, mybir
from concourse.bass_utils import run_bass_kernel_spmd

F32 = mybir.dt.float32
BF16 = mybir.dt.bfloat16
I16 = mybir.dt.int16
AF = mybir.ActivationFunctionType
ALU = mybir.AluOpType
BF_NP = ml_dtypes.bfloat16


class Cfg:
    def __init__(self, N, E, F, H, C, R, ncores=8, B=32, J=8):
        self.N, self.E, self.F, self.H, self.C, self.R = N, E, F, H, C, R
        self.ncores = ncores
        self.P = 128
        # node slice per core, multiple of 128 so dest tiles align to cores
        self.NLOC = math.ceil(N / ncores / 128) * 128
        self.MC = self.NLOC // 128                   # dest tiles per core
        self.MPAD = self.NLOC                        # padded nodes per core
        self.N_PAD = self.NLOC * ncores
        self.NT = self.N_PAD // 128                  # total dest tiles
        self.DUMMY = self.MPAD * R                   # zero row in tables
        self.B = B                                   # chunks per gather batch
        self.KC = F // 128                           # k-chunks layer-1 dense
        assert F % 128 == 0 and H == 128
        assert self.MPAD * R + 128 < 32768, "int16 gather index overflow"


CFG = Cfg(N=50000, E=800000, F=256, H=128, C=16, R=4, B=int(os.environ.get("KB", "8")))


def _bf(a):
    return np.ascontiguousarray(np.asarray(a, dtype=np.float32).astype(BF_NP))


# ----------------------------------------------------------------- host side
def preprocess(cfg, x, edge_index, edge_relation, W1, b1, W2, b2):
    N, ncores, NLOC, NT, R, MC = (cfg.N, cfg.ncores, cfg.NLOC, cfg.NT,
                                  cfg.R, cfg.MC)
    row = np.asarray(edge_index[0], dtype=np.int64)
    col = np.asarray(edge_index[1], dtype=np.int64)
    rel = np.asarray(edge_relation, dtype=np.int64)
    x = np.asarray(x, dtype=np.float32)
    W1 = np.asarray(W1, dtype=np.float32)
    b1 = np.asarray(b1, dtype=np.float32)
    W2 = np.asarray(W2, dtype=np.float32)
    b2 = np.asarray(b2, dtype=np.float32)

    deg = np.bincount(row, minlength=N).astype(np.float32)
    dinv = np.where(deg > 0, 1.0 / np.sqrt(np.maximum(deg, 1.0)),
                    0.0).astype(np.float32)

    # per-core edge sets (by source/col ownership), sorted by dest row
    per_core = []
    counts = np.zeros((ncores, NT), dtype=np.int64)
    for k in range(ncores):
        m = (col // NLOC) == k
        er, ec, eg = row[m], col[m] - k * NLOC, rel[m]
        o = np.argsort(er, kind="stable")
        er, ec, eg = er[o], ec[o], eg[o]
        t = er // 128
        counts[k] = np.bincount(t, minlength=NT)
        per_core.append((er, ec, eg, t))

    chunks_t = np.maximum(1, np.ceil(counts.max(axis=0) / 128).astype(np.int64))
    CH = int(chunks_t.sum())
    CHpad = math.ceil(CH / cfg.B) * cfg.B
    NB = CHpad // cfg.B
    slot_base = np.concatenate([[0], np.cumsum(chunks_t * 128)])[:-1]

    # shared weight packs
    #   w1cat[kc, k, r*H + h] = W1[r*F + kc*128 + k, h]
    w1cat = np.zeros((cfg.KC, 128, R * cfg.H), dtype=np.float32)
    for r in range(R):
        for kc in range(cfg.KC):
            w1cat[kc, :, r * cfg.H:(r + 1) * cfg.H] = \
                W1[r * cfg.F + kc * 128: r * cfg.F + (kc + 1) * 128, :]
    w1cat = _bf(w1cat.reshape(cfg.KC * 128, R * cfg.H))
    #   w2cat[h, r*C + c] = W2[r*H + h, c]
    w2cat = np.zeros((cfg.H, R * cfg.C), dtype=np.float32)
    for r in range(R):
        w2cat[:, r * cfg.C:(r + 1) * cfg.C] = W2[r * cfg.H:(r + 1) * cfg.H, :]
    w2cat = _bf(w2cat)
    #   crow[p, r*C + c] = (b1 @ W2_r)[c]
    crow1 = np.concatenate([b1 @ W2[r * cfg.H:(r + 1) * cfg.H, :]
                            for r in range(R)])
    crow = np.broadcast_to(crow1.astype(np.float32), (128, R * cfg.C)).copy()
    iota2 = _bf(np.broadcast_to(np.arange(128, dtype=np.float32), (128, 128)))
    b2r = np.broadcast_to(b2, (128, cfg.C)).astype(np.float32).copy()

    in_maps = []
    for k in range(ncores):
        er, ec, eg, t = per_core[k]
        first = np.searchsorted(t, np.arange(NT), side="left")
        rank = np.arange(len(t)) - first[t]
        slots = slot_base[t] + rank
        gidx = np.full(CHpad * 128, cfg.DUMMY, dtype=np.int16)
        gidx[slots] = (ec * R + eg).astype(np.int16)
        dloc = np.zeros(CHpad * 128, dtype=np.float32)
        dloc[slots] = (er % 128).astype(np.float32)

        # wrapped-16 int16 index layout, replicated to 8 Q7 groups, then
        # blocked so the whole table lives in one [128, NB*B*8] SBUF tile
        g = gidx.reshape(NB, cfg.B * 8, 16)
        w = np.transpose(g, (0, 2, 1))               # [b, 16, B*8]
        gidx_w = np.broadcast_to(
            w[:, None, :, :], (NB, 8, 16, cfg.B * 8)
        ).reshape(NB, 128, cfg.B * 8)
        gidx_sb = np.ascontiguousarray(
            np.transpose(gidx_w, (1, 0, 2)).reshape(128, NB * cfg.B * 8))
        dloc_sb = np.ascontiguousarray(dloc.reshape(CHpad, 128).T)

        lo = k * NLOC
        hi = min(N, lo + NLOC)
        nk = hi - lo
        uk = np.zeros((cfg.MPAD, cfg.F), dtype=np.float32)
        uk[:nk] = x[lo:hi] * dinv[lo:hi, None]
        dk = np.zeros(cfg.MPAD, dtype=np.float32)
        dk[:nk] = dinv[lo:hi]

        dinv_c = np.ascontiguousarray(dk.reshape(MC, 128).T)      # [128, MC]
        dinv2_r = _bf(np.broadcast_to(dk * dk, (128, cfg.MPAD)))  # [128, MC*128]
        dinvcr = np.ascontiguousarray(
            np.repeat(dk.reshape(MC, 128), cfg.C, axis=1)
            .reshape(MC, 128, cfg.C).transpose(1, 0, 2)
            .reshape(128, MC * cfg.C))                            # [128, MC*C]

        in_maps.append({
            "uT": _bf(uk.T),                     # [F, MPAD]
            "w1cat": w1cat,                      # [KC*128, R*H]
            "w2cat": w2cat,                      # [H, R*C]
            "crow": crow,                        # [128, R*C] f32
            "iota2": iota2,                      # [128, 128]
            "b2r": b2r,                          # [128, C] f32
            "gidx": gidx_sb,                     # [128, NB*B*8] i16
            "dloc": dloc_sb,                     # [128, CHpad]
            "dinvc": dinv_c.astype(np.float32),  # [128, MC] f32
            "dinv2r": dinv2_r,                   # [128, MC*128]
            "dinvcr": dinvcr.astype(np.float32),  # [128, MC*C] f32
        })
    return in_maps, tuple(int(v) for v in chunks_t), CHpad


# --------------------------------------------------------------- device side
def build_program(cfg, chunks_t, CHpad):
    R, H, C = cfg.R, cfg.H, cfg.C
    NB = CHpad // cfg.B
    nc = bacc.Bacc("TRN2", target_bir_lowering=False, debug=False,
                   num_devices=cfg.ncores)

    uT = nc.dram_tensor("uT", [cfg.F, cfg.MPAD], BF16, kind="ExternalInput").ap()
    w1cat = nc.dram_tensor("w1cat", [cfg.KC * 128, R * H], BF16,
                           kind="ExternalInput").ap()
    w2cat = nc.dram_tensor("w2cat", [H, R * C], BF16, kind="ExternalInput").ap()
    crow = nc.dram_tensor("crow", [128, R * C], F32, kind="ExternalInput").ap()
    iota2 = nc.dram_tensor("iota2", [128, 128], BF16, kind="ExternalInput").ap()
    b2r = nc.dram_tensor("b2r", [128, C], F32, kind="ExternalInput").ap()
    gidx = nc.dram_tensor("gidx", [128, NB * cfg.B * 8], I16,
                          kind="ExternalInput").ap()
    dloc = nc.dram_tensor("dloc", [128, CHpad], F32, kind="ExternalInput").ap()
    dinvc = nc.dram_tensor("dinvc", [128, cfg.MC], F32,
                           kind="ExternalInput").ap()
    dinv2r = nc.dram_tensor("dinv2r", [128, cfg.MC * 128], BF16,
                            kind="ExternalInput").ap()
    dinvcr = nc.dram_tensor("dinvcr", [128, cfg.MC * C], F32,
                            kind="ExternalInput").ap()
    outT = nc.dram_tensor("outT", [128, cfg.MC * C], F32,
                          kind="ExternalOutput").ap()

    with tile.TileContext(nc) as tc:
        _build(tc, cfg, chunks_t, CHpad, uT, w1cat, w2cat, crow, iota2, b2r,
               gidx, dloc, dinvc, dinv2r, dinvcr, outT)
    nc.compile()
    return nc


def _build(tc, cfg, chunks_t, CHpad, uT, w1cat, w2cat, crow, iota2, b2r,
           gidx, dloc, dinvc, dinv2r, dinvcr, outT):
    nc = tc.nc
    R, H, C, B, MC, NT, KC = (cfg.R, cfg.H, cfg.C, cfg.B, cfg.MC, cfg.NT,
                              cfg.KC)
    NB = CHpad // B
    NCORES = cfg.ncores
    TROWS = cfg.MPAD * R                      # real table rows
    with tc.tile_pool(name="const", bufs=1) as cpool, \
         tc.tile_pool(name="big", bufs=1) as bigp, \
         tc.tile_pool(name="stg", bufs=2) as stgp, \
         tc.tile_pool(name="gY", bufs=3) as gpool, \
         tc.tile_pool(name="s3", bufs=12) as spool, \
         tc.tile_pool(name="ev", bufs=4) as evp, \
         tc.tile_pool(name="psA", bufs=2, space="PSUM") as ppa, \
         tc.tile_pool(name="psB", bufs=6, space="PSUM") as ppb, \
         tc.tile_pool(name="dram", bufs=1, space="DRAM") as dram:

        # ---------- resident inputs
        iota_t = cpool.tile([128, 128], BF16)
        nc.sync.dma_start(out=iota_t[:], in_=iota2[:, :])
        crow_t = cpool.tile([128, R * C], F32)
        nc.sync.dma_start(out=crow_t[:], in_=crow[:, :])
        b2_t = cpool.tile([128, C], F32)
        nc.sync.dma_start(out=b2_t[:], in_=b2r[:, :])
        dinvc_t = cpool.tile([128, MC], F32)
        nc.sync.dma_start(out=dinvc_t[:], in_=dinvc[:, :])
        w1_t = cpool.tile([128, KC, R * H], BF16)
        nc.sync.dma_start(
            out=w1_t[:],
            in_=w1cat.rearrange("(kc p) n -> p kc n", kc=KC, p=128))
        w2_t = cpool.tile([128, R * C], BF16)
        nc.sync.dma_start(out=w2_t[:], in_=w2cat[:, :])
        gidx_t = bigp.tile([128, NB * B * 8], I16)
        nc.sync.dma_start(out=gidx_t[:], in_=gidx[:, :])
        dloc_t = bigp.tile([128, CHpad], F32)
        nc.sync.dma_start(out=dloc_t[:], in_=dloc[:, :])
        uT_t = bigp.tile([128, KC, cfg.MPAD], BF16)
        nc.sync.dma_start(
            out=uT_t[:],
            in_=uT.rearrange("(kc p) n -> p kc n", kc=KC, p=128))
        zrow = cpool.tile([128, 128], BF16)
        nc.vector.memset(zrow[:], 0.0)

        # shared table staging [128, MC*R*H] (y1: all cols; y2: :C per block)
        stage = bigp.tile([128, MC * R * H], BF16)

        # DRAM tensors
        y1_dram = dram.tile([TROWS + 128, H], BF16)
        y2_dram = dram.tile([TROWS + 128, H], BF16)
        t1_part = dram.tile([NCORES * 128, MC * 128], BF16)
        t1_red = dram.tile([128, MC * 128], BF16)
        t2_part = dram.tile([NCORES * 128, MC * C], BF16)
        t2_red = dram.tile([128, MC * C], BF16)

        # ---------- layer-1 dense: stage[p, mc*512 + r*H + h]
        for mc in range(MC):
            ps = ppa.tile([128, R * H], F32, tag="psA")
            for kc in range(KC):
                nc.tensor.matmul(
                    out=ps[:],
                    lhsT=uT_t[:, kc, mc * 128:(mc + 1) * 128],
                    rhs=w1_t[:, kc, :],
                    start=(kc == 0), stop=(kc == KC - 1))
            nc.scalar.copy(out=stage[:, mc * R * H:(mc + 1) * R * H],
                           in_=ps[:])
        nc.sync.dma_start(
            out=y1_dram[0:TROWS, :].rearrange(
                "(mc p r) h -> p mc (r h)", mc=MC, p=128, r=R),
            in_=stage[:])
        nc.sync.dma_start(out=y1_dram[TROWS:TROWS + 128, :], in_=zrow[:])

        LIMIT = int(os.environ.get("KLIMIT", "6"))
        if LIMIT < 2:
            return

        # ---------- shared gather + one-hot segment-sum pass
        def agg_pass(table_ap, width, part_dram, transposed, tagsfx):
            batches = {}

            def batch(b):
                if b not in batches:
                    g = gpool.tile([128, B, H], BF16, tag="g")
                    nc.gpsimd.dma_gather(
                        out_ap=g[:], in_ap=table_ap,
                        idxs_ap=gidx_t[:, b * B * 8:(b + 1) * B * 8],
                        num_idxs=B * 128, num_idxs_reg=B * 128,
                        elem_size=H)
                    batches[b] = g
                return batches[b]

            c = 0
            st = None
            for t in range(NT):
                k, j = t // MC, t % MC
                if j == 0:
                    st = stgp.tile([128, MC * width], BF16, tag=f"st{tagsfx}")
                ps = ppb.tile([128, width], F32, tag="ps")
                for jj in range(chunks_t[t]):
                    g = batch(c // B)
                    s3 = spool.tile([128, 128], BF16, tag="s3")
                    nc.vector.tensor_scalar(
                        out=s3[:], in0=iota_t[:], scalar1=dloc_t[:, c:c + 1],
                        scalar2=None, op0=ALU.is_equal)
                    if transposed:
                        nc.tensor.matmul(
                            out=ps[:], lhsT=g[:, c % B, :], rhs=s3[:],
                            start=(jj == 0), stop=(jj == chunks_t[t] - 1))
                    else:
                        nc.tensor.matmul(
                            out=ps[:], lhsT=s3[:], rhs=g[:, c % B, :width],
                            start=(jj == 0), stop=(jj == chunks_t[t] - 1))
                    c += 1
                nc.scalar.copy(out=st[:, j * width:(j + 1) * width], in_=ps[:])
                if j == MC - 1:
                    nc.sync.dma_start(
                        out=part_dram[k * 128:(k + 1) * 128, :], in_=st[:])

        # ---------- layer-1 aggregation (transposed) + reduce-scatter
        agg_pass(y1_dram[:, :], 128, t1_part, True, "1")
        if LIMIT < 3:
            return
        nc.gpsimd.collective_compute(
            "ReduceScatter", ALU.add,
            replica_groups=[list(range(NCORES))],
            ins=[t1_part.opt()], outs=[t1_red.opt()])
        if LIMIT < 4:
            return

        # ---------- layer-2 dense: u2T = t1_red * dinv2 ; y2 = u2 @ W2cat
        dinv2_t = bigp.tile([128, MC * 128], BF16)
        nc.sync.dma_start(out=dinv2_t[:], in_=dinv2r[:, :])
        t1r_t = bigp.tile([128, MC * 128], BF16)
        nc.sync.dma_start(out=t1r_t[:], in_=t1_red[:, :])
        u2T_t = bigp.tile([128, MC * 128], BF16)
        nc.vector.tensor_tensor(out=u2T_t[:], in0=t1r_t[:], in1=dinv2_t[:],
                                op=ALU.mult)
        # wipe the staging buffer (y2 rows are C-wide in 128-wide blocks)
        nc.vector.memset(stage[:], 0.0)
        for mc in range(MC):
            ps = ppa.tile([128, R * C], F32, tag="psA")
            nc.tensor.matmul(out=ps[:],
                             lhsT=u2T_t[:, mc * 128:(mc + 1) * 128],
                             rhs=w2_t[:], start=True, stop=True)
            bias = evp.tile([128, R * C], F32, tag="bias")
            nc.vector.tensor_scalar(out=bias[:], in0=crow_t[:],
                                    scalar1=dinvc_t[:, mc:mc + 1],
                                    scalar2=None, op0=ALU.mult)
            nc.vector.tensor_tensor(
                out=stage[:, mc * R * H:(mc + 1) * R * H]
                    .rearrange("p (r h) -> p r h", r=R)[:, :, 0:C],
                in0=ps[:].rearrange("p (r c) -> p r c", r=R),
                in1=bias[:].rearrange("p (r c) -> p r c", r=R),
                op=ALU.add)
        nc.sync.dma_start(
            out=y2_dram[0:TROWS, :].rearrange(
                "(mc p r) h -> p mc (r h)", mc=MC, p=128, r=R),
            in_=stage[:])
        nc.sync.dma_start(out=y2_dram[TROWS:TROWS + 128, :], in_=zrow[:])
        if LIMIT < 5:
            return

        # ---------- layer-2 aggregation (node-major) + reduce-scatter
        agg_pass(y2_dram[:, :], C, t2_part, False, "2")
        nc.gpsimd.collective_compute(
            "ReduceScatter", ALU.add,
            replica_groups=[list(range(NCORES))],
            ins=[t2_part.opt()], outs=[t2_red.opt()])
        if LIMIT < 6:
            return

        # ---------- final: h2 = dinv*t2 + b2 ; log_softmax over C
        dinvcr_t = bigp.tile([128, MC * C], F32)
        nc.sync.dma_start(out=dinvcr_t[:], in_=dinvcr[:, :])
        t2r_t = bigp.tile([128, MC * C], BF16)
        nc.sync.dma_start(out=t2r_t[:], in_=t2_red[:, :])
        h2 = bigp.tile([128, MC, C], F32)
        nc.vector.tensor_tensor(
            out=h2[:], in0=t2r_t[:].rearrange("p (m c) -> p m c", c=C),
            in1=dinvcr_t[:].rearrange("p (m c) -> p m c", c=C), op=ALU.mult)
        nc.vector.tensor_tensor(
            out=h2[:], in0=h2[:],
            in1=b2_t[:].unsqueeze(1).to_broadcast([128, MC, C]), op=ALU.add)
        negmx = bigp.tile([128, MC, 1], F32)
        nc.vector.tensor_reduce(out=negmx[:], in_=h2[:],
                                axis=mybir.AxisListType.X,
                                op=ALU.max, negate=True)
        nc.vector.tensor_tensor(
            out=h2[:], in0=h2[:], in1=negmx[:].to_broadcast([128, MC, C]),
            op=ALU.add)
        ex = bigp.tile([128, MC, C], F32)
        nc.scalar.activation(out=ex[:], in_=h2[:], func=AF.Exp)
        ssum = bigp.tile([128, MC, 1], F32)
        nc.vector.tensor_reduce(out=ssum[:], in_=ex[:],
                                axis=mybir.AxisListType.X, op=ALU.add)
        lg = bigp.tile([128, MC, 1], F32)
        nc.scalar.activation(out=lg[:], in_=ssum[:], func=AF.Ln)
        nc.vector.tensor_tensor(
            out=h2[:], in0=h2[:], in1=lg[:].to_broadcast([128, MC, C]),
            op=ALU.subtract)
        nc.sync.dma_start(
            out=outT[:, :], in_=h2[:].rearrange("p m c -> p (m c)"))


# ------------------------------------------------------------------ runtime
_PROGRAM_CACHE = {}


def run(cfg, inputs):
    in_maps, chunks_t, CHpad = preprocess(cfg, **inputs)
    key = (cfg.N, cfg.E, chunks_t, CHpad)
    if key not in _PROGRAM_CACHE:
        _PROGRAM_CACHE[key] = build_program(cfg, chunks_t, CHpad)
    nc = _PROGRAM_CACHE[key]
    res = None
    for attempt in range(3):
        try:
            res = run_bass_kernel_spmd(nc, in_maps,
                                       core_ids=list(range(cfg.ncores)))
            break
        except Exception:
            if attempt == 2:
                raise
    outs = []
    for k in range(cfg.ncores):
        oT = res.results[k]["outT"]                       # [128, MC*C]
        o = oT.reshape(128, cfg.MC, cfg.C).transpose(1, 0, 2).reshape(
            cfg.NLOC, cfg.C)
        outs.append(o)
    full = np.concatenate(outs, axis=0)[:cfg.N]
    return np.ascontiguousarray(full.astype(np.float32))


def kernel(x, edge_index, edge_relation, W1, b1, W2, b2):
    return run(CFG, dict(x=x, edge_index=edge_index,
                         edge_relation=edge_relation,
                         W1=W1, b1=b1, W2=W2, b2=b2))


# revision 8
# speedup vs baseline: 1.5428x; 1.0386x over previous
"""GeomGCN (2-layer relational GCN) distributed Bass kernel for 8 TRN2 NeuronCores.

Strategy (source-sharded, graph-parallel, bf16 hot path):
  - Nodes split into 8 slices of NLOC (multiple of 128); core k owns slice k
    and all edges whose source `col` lies in it.
  - Message tables are (node, rel)-combined: row index = node*4 + rel, so a
    node's 4 relation messages are 1KB-contiguous and each table is written
    with ONE fat DMA.  Both layers share one gather-index table
    (idx = col_local*4 + rel), resident in SBUF.
  - Gather slots use grouped continuous packing: per group of G dest tiles,
    each core packs its edges continuously into shared 128-slot blocks; the
    static schedule is a list of (block, tile) chunks whose one-hot S is
    built per chunk from a chunk-indexed dloc column (sentinel -1 for
    foreign/dummy slots).  This cuts dummy-slot gather waste from ~50% to
    ~6%.
  - Layer-1 aggregation is TRANSPOSED on TensorE (lhsT = gathered messages,
    rhs = S) so partials come out [H, node]; each dest tile is then
    immediately multiplied by W2cat on-core ((D t1)@W2 = D (t1@W2)), so the
    ReduceScatter moves 64-wide y2 partials and the layer-2 dense phase
    disappears.  dinv^2 scaling + b1-path bias are applied post-RS while
    assembling the layer-2 table.
  - Layer-2 aggregation is node-major (lhsT = S), 16-wide; after a second
    ReduceScatter a fused log_softmax (2 activation table loads total)
    emits a transposed [128, MC*16] output that the host un-transposes.
  Host work: index prep, degree/dinv, pre-scaling x by dinv, bf16 packing.
"""
import math
import os
import numpy as np
import ml_dtypes

import concourse.bass as bass
import concourse.tile as tile
from concourse import bacc, mybir
from concourse.bass_utils import run_bass_kernel_spmd

F32 = mybir.dt.float32
BF16 = mybir.dt.bfloat16
I16 = mybir.dt.int16
AF = mybir.ActivationFunctionType
ALU = mybir.AluOpType
BF_NP = ml_dtypes.bfloat16


class Cfg:
    def __init__(self, N, E, F, H, C, R, ncores=8, B=8, G=8, J=8):
        self.N, self.E, self.F, self.H, self.C, self.R = N, E, F, H, C, R
        self.ncores = ncores
        self.P = 128
        # node slice per core, multiple of 128 so dest tiles align to cores
        self.NLOC = math.ceil(N / ncores / 128) * 128
        self.MC = self.NLOC // 128                   # dest tiles per core
        self.MPAD = self.NLOC                        # padded nodes per core
        self.N_PAD = self.NLOC * ncores
        self.NT = self.N_PAD // 128                  # total dest tiles
        self.DUMMY = self.MPAD * R                   # zero row in tables
        self.B = B                                   # blocks per gather batch
        self.G = G                                   # tiles per packing group
        self.KC = F // 128                           # k-chunks layer-1 dense
        assert F % 128 == 0 and H == 128
        assert self.MPAD * R + 128 < 32768, "int16 gather index overflow"


CFG = Cfg(N=50000, E=800000, F=256, H=128, C=16, R=4,
          B=int(os.environ.get("KB", "8")), G=int(os.environ.get("KG", "8")))


def _bf(a):
    return np.ascontiguousarray(np.asarray(a, dtype=np.float32).astype(BF_NP))


# ----------------------------------------------------------------- host side
def preprocess(cfg, x, edge_index, edge_relation, W1, b1, W2, b2):
    N, ncores, NLOC, NT, R, MC, G = (cfg.N, cfg.ncores, cfg.NLOC, cfg.NT,
                                     cfg.R, cfg.MC, cfg.G)
    row = np.asarray(edge_index[0], dtype=np.int64)
    col = np.asarray(edge_index[1], dtype=np.int64)
    rel = np.asarray(edge_relation, dtype=np.int64)
    x = np.asarray(x, dtype=np.float32)
    W1 = np.asarray(W1, dtype=np.float32)
    b1 = np.asarray(b1, dtype=np.float32)
    W2 = np.asarray(W2, dtype=np.float32)
    b2 = np.asarray(b2, dtype=np.float32)

    deg = np.bincount(row, minlength=N).astype(np.float32)
    dinv = np.where(deg > 0, 1.0 / np.sqrt(np.maximum(deg, 1.0)),
                    0.0).astype(np.float32)

    # per-core edge sets (by source/col ownership), sorted by dest row
    per_core = []
    counts = np.zeros((ncores, NT), dtype=np.int64)
    for k in range(ncores):
        m = (col // NLOC) == k
        er, ec, eg = row[m], col[m] - k * NLOC, rel[m]
        o = np.argsort(er, kind="stable")
        er, ec, eg = er[o], ec[o], eg[o]
        counts[k] = np.bincount(er // 128, minlength=NT)
        per_core.append((er, ec, eg))

    # ---- grouped continuous packing (shared static schedule)
    # groups of up to G tiles, never crossing a dest-core boundary
    groups = []
    for dk in range(ncores):
        t0 = dk * MC
        for g0 in range(0, MC, G):
            groups.append((t0 + g0, t0 + min(MC, g0 + G)))
    gi_of_tile = np.zeros(NT, dtype=np.int64)
    for gi, (tlo, thi) in enumerate(groups):
        gi_of_tile[tlo:thi] = gi
    csum = [np.concatenate([[0], np.cumsum(counts[k])]) for k in range(ncores)]
    tiles_chunks = [[] for _ in range(NT)]   # per tile: list of block ids
    SB = 0
    core_gbase = np.zeros((ncores, len(groups)), dtype=np.int64)
    for gi, (tlo, thi) in enumerate(groups):
        nblk = 1
        for k in range(ncores):
            core_gbase[k, gi] = SB * 128
            sgk = int(csum[k][thi] - csum[k][tlo])
            nblk = max(nblk, math.ceil(sgk / 128))
        for t in range(tlo, thi):
            lo, hi = None, None
            for k in range(ncores):
                p0 = int(csum[k][t] - csum[k][tlo])
                p1 = int(csum[k][t + 1] - csum[k][tlo])
                if p1 > p0:
                    l, h = p0 // 128, (p1 - 1) // 128
                    lo = l if lo is None else min(lo, l)
                    hi = h if hi is None else max(hi, h)
            if lo is None:
                lo = hi = 0
            tiles_chunks[t] = [SB + b for b in range(lo, hi + 1)]
        SB += nblk
    NBAT = math.ceil(SB / cfg.B)
    SBpad = NBAT * cfg.B
    # chunk order = tile-major; chunk index per (tile, block)
    chunk_of = {}
    CH = 0
    for t in range(NT):
        for b in tiles_chunks[t]:
            chunk_of[(t, b)] = CH
            CH += 1

    # shared weight packs
    w1cat = np.zeros((cfg.KC, 128, R * cfg.H), dtype=np.float32)
    for r in range(R):
        for kc in range(cfg.KC):
            w1cat[kc, :, r * cfg.H:(r + 1) * cfg.H] = \
                W1[r * cfg.F + kc * 128: r * cfg.F + (kc + 1) * 128, :]
    w1cat = _bf(w1cat.reshape(cfg.KC * 128, R * cfg.H))
    w2cat = np.zeros((cfg.H, R * cfg.C), dtype=np.float32)
    for r in range(R):
        w2cat[:, r * cfg.C:(r + 1) * cfg.C] = W2[r * cfg.H:(r + 1) * cfg.H, :]
    w2cat = _bf(w2cat)
    crow1 = np.concatenate([b1 @ W2[r * cfg.H:(r + 1) * cfg.H, :]
                            for r in range(R)])
    crow = np.broadcast_to(crow1.astype(np.float32), (128, R * cfg.C)).copy()
    iota2 = _bf(np.broadcast_to(np.arange(128, dtype=np.float32), (128, 128)))
    b2r = np.broadcast_to(b2, (128, cfg.C)).astype(np.float32).copy()

    in_maps = []
    for k in range(ncores):
        er, ec, eg = per_core[k]
        t = er // 128
        egi = gi_of_tile[t]
        # position within group = running index of edge within its group
        first_of_group = np.concatenate(
            [[0], np.cumsum(np.bincount(egi, minlength=len(groups)))])[:-1]
        pos_in_group = np.arange(len(er)) - first_of_group[egi]
        slots = core_gbase[k][egi] + pos_in_group

        gidx = np.full(SBpad * 128, cfg.DUMMY, dtype=np.int16)
        gidx[slots] = (ec * R + eg).astype(np.int16)
        # chunk-indexed dloc: for chunk (t, b): slot p of block b ->
        #   er%128 if the edge belongs to tile t else -1
        dloc = np.full((CH, 128), -1.0, dtype=np.float32)
        eb = slots // 128
        ep = slots % 128
        eci = np.fromiter((chunk_of[(int(tt), int(bb))]
                           for tt, bb in zip(t, eb)),
                          dtype=np.int64, count=len(er))
        dloc[eci, ep] = (er % 128).astype(np.float32)

        # wrapped-16 int16 index layout, replicated to 8 Q7 groups, blocked
        # into one [128, NBAT*B*8] resident SBUF tile
        gg = gidx.reshape(NBAT, cfg.B * 8, 16)
        w = np.transpose(gg, (0, 2, 1))
        gidx_w = np.broadcast_to(
            w[:, None, :, :], (NBAT, 8, 16, cfg.B * 8)
        ).reshape(NBAT, 128, cfg.B * 8)
        gidx_sb = np.ascontiguousarray(
            np.transpose(gidx_w, (1, 0, 2)).reshape(128, NBAT * cfg.B * 8))
        dloc_sb = np.ascontiguousarray(dloc.T)       # [128, CH]

        lo = k * NLOC
        hi = min(N, lo + NLOC)
        nk = hi - lo
        uk = np.zeros((cfg.MPAD, cfg.F), dtype=np.float32)
        uk[:nk] = x[lo:hi] * dinv[lo:hi, None]
        dk = np.zeros(cfg.MPAD, dtype=np.float32)
        dk[:nk] = dinv[lo:hi]

        dinv_c = np.ascontiguousarray(dk.reshape(MC, 128).T)      # [128, MC]
        dinvcr = np.ascontiguousarray(
            np.repeat(dk.reshape(MC, 128), cfg.C, axis=1)
            .reshape(MC, 128, cfg.C).transpose(1, 0, 2)
            .reshape(128, MC * cfg.C))                            # [128, MC*C]

        in_maps.append({
            "uT": _bf(uk.T),                     # [F, MPAD]
            "w1cat": w1cat,                      # [KC*128, R*H]
            "w2cat": w2cat,                      # [H, R*C]
            "crow": crow,                        # [128, R*C] f32
            "iota2": iota2,                      # [128, 128]
            "b2r": b2r,                          # [128, C] f32
            "gidx": gidx_sb,                     # [128, NBAT*B*8] i16
            "dloc": dloc_sb,                     # [128, CH] f32
            "dinvc": dinv_c.astype(np.float32),  # [128, MC] f32
            "dinvcr": dinvcr.astype(np.float32),  # [128, MC*C] f32
        })
    sched = tuple(tuple(tc_) for tc_ in tiles_chunks)
    return in_maps, sched, SBpad


# --------------------------------------------------------------- device side
def build_program(cfg, sched, SBpad):
    R, H, C = cfg.R, cfg.H, cfg.C
    NBAT = SBpad // cfg.B
    CH = sum(len(s) for s in sched)
    nc = bacc.Bacc("TRN2", target_bir_lowering=False, debug=False,
                   num_devices=cfg.ncores)

    uT = nc.dram_tensor("uT", [cfg.F, cfg.MPAD], BF16, kind="ExternalInput").ap()
    w1cat = nc.dram_tensor("w1cat", [cfg.KC * 128, R * H], BF16,
                           kind="ExternalInput").ap()
    w2cat = nc.dram_tensor("w2cat", [H, R * C], BF16, kind="ExternalInput").ap()
    crow = nc.dram_tensor("crow", [128, R * C], F32, kind="ExternalInput").ap()
    iota2 = nc.dram_tensor("iota2", [128, 128], BF16, kind="ExternalInput").ap()
    b2r = nc.dram_tensor("b2r", [128, C], F32, kind="ExternalInput").ap()
    gidx = nc.dram_tensor("gidx", [128, NBAT * cfg.B * 8], I16,
                          kind="ExternalInput").ap()
    dloc = nc.dram_tensor("dloc", [128, CH], F32, kind="ExternalInput").ap()
    dinvc = nc.dram_tensor("dinvc", [128, cfg.MC], F32,
                           kind="ExternalInput").ap()
    dinvcr = nc.dram_tensor("dinvcr", [128, cfg.MC * C], F32,
                            kind="ExternalInput").ap()
    outT = nc.dram_tensor("outT", [128, cfg.MC * C], F32,
                          kind="ExternalOutput").ap()

    with tile.TileContext(nc) as tc:
        _build(tc, cfg, sched, SBpad, uT, w1cat, w2cat, crow, iota2, b2r,
               gidx, dloc, dinvc, dinvcr, outT)
    nc.compile()
    return nc


def _build(tc, cfg, sched, SBpad, uT, w1cat, w2cat, crow, iota2, b2r,
           gidx, dloc, dinvc, dinvcr, outT):
    nc = tc.nc
    R, H, C, B, MC, NT, KC = (cfg.R, cfg.H, cfg.C, cfg.B, cfg.MC, cfg.NT,
                              cfg.KC)
    NBAT = SBpad // B
    NCORES = cfg.ncores
    TROWS = cfg.MPAD * R
    CH = sum(len(s) for s in sched)
    RC = R * C
    with tc.tile_pool(name="const", bufs=1) as cpool, \
         tc.tile_pool(name="big", bufs=1) as bigp, \
         tc.tile_pool(name="stg", bufs=2) as stgp, \
         tc.tile_pool(name="gY", bufs=3) as gpool, \
         tc.tile_pool(name="s3", bufs=12) as spool, \
         tc.tile_pool(name="ev", bufs=4) as evp, \
         tc.tile_pool(name="psA", bufs=2, space="PSUM") as ppa, \
         tc.tile_pool(name="psB", bufs=6, space="PSUM") as ppb, \
         tc.tile_pool(name="dram", bufs=1, space="DRAM") as dram:

        # ---------- resident inputs (dense-phase operands first)
        uT_t = bigp.tile([128, KC, cfg.MPAD], BF16)
        nc.sync.dma_start(
            out=uT_t[:],
            in_=uT.rearrange("(kc p) n -> p kc n", kc=KC, p=128))
        w1_t = cpool.tile([128, KC, R * H], BF16)
        nc.sync.dma_start(
            out=w1_t[:],
            in_=w1cat.rearrange("(kc p) n -> p kc n", kc=KC, p=128))
        w2_t = cpool.tile([128, RC], BF16)
        nc.sync.dma_start(out=w2_t[:], in_=w2cat[:, :])
        iota_t = cpool.tile([128, 128], BF16)
        nc.sync.dma_start(out=iota_t[:], in_=iota2[:, :])
        gidx_t = bigp.tile([128, NBAT * B * 8], I16)
        nc.sync.dma_start(out=gidx_t[:], in_=gidx[:, :])
        dloc_t = bigp.tile([128, CH], F32)
        nc.sync.dma_start(out=dloc_t[:], in_=dloc[:, :])
        crow_t = cpool.tile([128, RC], F32)
        nc.sync.dma_start(out=crow_t[:], in_=crow[:, :])
        b2_t = cpool.tile([128, C], F32)
        nc.sync.dma_start(out=b2_t[:], in_=b2r[:, :])
        dinvc_t = cpool.tile([128, MC], F32)
        nc.sync.dma_start(out=dinvc_t[:], in_=dinvc[:, :])
        zrow = cpool.tile([128, 128], BF16)
        nc.vector.memset(zrow[:], 0.0)

        # shared table staging [128, MC*R*H] (y1: all cols; y2: :C per block)
        stage = bigp.tile([128, MC * R * H], BF16)

        # DRAM tensors
        y1_dram = dram.tile([TROWS + 128, H], BF16)
        y2_dram = dram.tile([TROWS + 128, H], BF16)
        y2_part = dram.tile([NCORES * 128, MC * RC], BF16)
        y2_red = dram.tile([128, MC * RC], BF16)
        t2_part = dram.tile([NCORES * 128, MC * C], BF16)
        t2_red = dram.tile([128, MC * C], BF16)

        # ---------- layer-1 dense: stage[p, mc*512 + r*H + h]
        for mc in range(MC):
            ps = ppa.tile([128, R * H], F32, tag="psA")
            for kc in range(KC):
                nc.tensor.matmul(
                    out=ps[:],
                    lhsT=uT_t[:, kc, mc * 128:(mc + 1) * 128],
                    rhs=w1_t[:, kc, :],
                    start=(kc == 0), stop=(kc == KC - 1))
            nc.scalar.copy(out=stage[:, mc * R * H:(mc + 1) * R * H],
                           in_=ps[:])
        nc.sync.dma_start(
            out=y1_dram[0:TROWS, :].rearrange(
                "(mc p r) h -> p mc (r h)", mc=MC, p=128, r=R),
            in_=stage[:])
        nc.sync.dma_start(out=y1_dram[TROWS:TROWS + 128, :], in_=zrow[:])

        LIMIT = int(os.environ.get("KLIMIT", "6"))
        if LIMIT < 2:
            return

        # ---------- shared gather + one-hot segment-sum pass
        def agg_pass(table_ap, width, part_dram, fuse_w2, tagsfx):
            batches = {}

            def batch(b):
                if b not in batches:
                    g = gpool.tile([128, B, H], BF16, tag="g")
                    nc.gpsimd.dma_gather(
                        out_ap=g[:], in_ap=table_ap,
                        idxs_ap=gidx_t[:, b * B * 8:(b + 1) * B * 8],
                        num_idxs=B * 128, num_idxs_reg=B * 128,
                        elem_size=H)
                    batches[b] = g
                return batches[b]

            ci = 0
            st = None
            for t in range(NT):
                k, j = t // MC, t % MC
                if j == 0:
                    st = stgp.tile([128, MC * width], BF16, tag=f"st{tagsfx}")
                ps = ppb.tile([128, 128 if fuse_w2 else width], F32, tag="ps")
                blocks = sched[t]
                for jj, b in enumerate(blocks):
                    g = batch(b // B)
                    s3 = spool.tile([128, 128], BF16, tag="s3")
                    nc.vector.tensor_scalar(
                        out=s3[:], in0=iota_t[:],
                        scalar1=dloc_t[:, ci:ci + 1],
                        scalar2=None, op0=ALU.is_equal)
                    if fuse_w2:
                        nc.tensor.matmul(
                            out=ps[:], lhsT=g[:, b % B, :], rhs=s3[:],
                            start=(jj == 0), stop=(jj == len(blocks) - 1))
                    else:
                        nc.tensor.matmul(
                            out=ps[:], lhsT=s3[:], rhs=g[:, b % B, :width],
                            start=(jj == 0), stop=(jj == len(blocks) - 1))
                    ci += 1
                if fuse_w2:
                    # t1T tile [h, node] -> (t1 @ W2cat) [node, RC]
                    tb = evp.tile([128, 128], BF16, tag="t1T")
                    nc.scalar.copy(out=tb[:], in_=ps[:])
                    ps2 = ppa.tile([128, RC], F32, tag="psA")
                    nc.tensor.matmul(out=ps2[:], lhsT=tb[:], rhs=w2_t[:],
                                     start=True, stop=True)
                    nc.scalar.copy(out=st[:, j * width:(j + 1) * width],
                                   in_=ps2[:])
                else:
                    nc.scalar.copy(out=st[:, j * width:(j + 1) * width],
                                   in_=ps[:])
                if j == MC - 1:
                    nc.sync.dma_start(
                        out=part_dram[k * 128:(k + 1) * 128, :], in_=st[:])

        # ---------- layer-1 aggregation fused with W2 + reduce-scatter
        agg_pass(y1_dram[:, :], RC, y2_part, True, "1")
        if LIMIT < 3:
            return
        nc.gpsimd.collective_compute(
            "ReduceScatter", ALU.add,
            replica_groups=[list(range(NCORES))],
            ins=[y2_part.opt()], outs=[y2_red.opt()])
        if LIMIT < 4:
            return

        # ---------- layer-2 table: y2 = dinv * (dinv * red + crow)
        y2r_t = bigp.tile([128, MC * RC], BF16)
        nc.sync.dma_start(out=y2r_t[:], in_=y2_red[:, :])
        for mc in range(MC):
            sc = evp.tile([128, RC], F32, tag="sc")
            nc.vector.tensor_scalar(out=sc[:],
                                    in0=y2r_t[:, mc * RC:(mc + 1) * RC],
                                    scalar1=dinvc_t[:, mc:mc + 1],
                                    scalar2=None, op0=ALU.mult)
            nc.vector.tensor_tensor(out=sc[:], in0=sc[:], in1=crow_t[:],
                                    op=ALU.add)
            nc.vector.tensor_scalar(
                out=stage[:, mc * R * H:(mc + 1) * R * H]
                    .rearrange("p (r h) -> p r h", r=R)[:, :, 0:C],
                in0=sc[:].rearrange("p (r c) -> p r c", r=R),
                scalar1=dinvc_t[:, mc:mc + 1],
                scalar2=None, op0=ALU.mult)
        nc.sync.dma_start(
            out=y2_dram[0:TROWS, :].rearrange(
                "(mc p r) h -> p mc (r h)", mc=MC, p=128, r=R),
            in_=stage[:])
        nc.sync.dma_start(out=y2_dram[TROWS:TROWS + 128, :], in_=zrow[:])
        if LIMIT < 5:
            return

        # ---------- layer-2 aggregation (node-major) + reduce-scatter
        agg_pass(y2_dram[:, :], C, t2_part, False, "2")
        nc.gpsimd.collective_compute(
            "ReduceScatter", ALU.add,
            replica_groups=[list(range(NCORES))],
            ins=[t2_part.opt()], outs=[t2_red.opt()])
        if LIMIT < 6:
            return

        # ---------- final: h2 = dinv*t2 + b2 ; log_softmax over C
        dinvcr_t = bigp.tile([128, MC * C], F32)
        nc.sync.dma_start(out=dinvcr_t[:], in_=dinvcr[:, :])
        t2r_t = bigp.tile([128, MC * C], BF16)
        nc.sync.dma_start(out=t2r_t[:], in_=t2_red[:, :])
        h2 = bigp.tile([128, MC, C], F32)
        nc.vector.tensor_tensor(
            out=h2[:], in0=t2r_t[:].rearrange("p (m c) -> p m c", c=C),
            in1=dinvcr_t[:].rearrange("p (m c) -> p m c", c=C), op=ALU.mult)
        nc.vector.tensor_tensor(
            out=h2[:], in0=h2[:],
            in1=b2_t[:].unsqueeze(1).to_broadcast([128, MC, C]), op=ALU.add)
        negmx = bigp.tile([128, MC, 1], F32)
        nc.vector.tensor_reduce(out=negmx[:], in_=h2[:],
                                axis=mybir.AxisListType.X,
                                op=ALU.max, negate=True)
        nc.vector.tensor_tensor(
            out=h2[:], in0=h2[:], in1=negmx[:].to_broadcast([128, MC, C]),
            op=ALU.add)
        ex = bigp.tile([128, MC, C], F32)
        nc.scalar.activation(out=ex[:], in_=h2[:], func=AF.Exp)
        ssum = bigp.tile([128, MC, 1], F32)
        nc.vector.tensor_reduce(out=ssum[:], in_=ex[:],
                                axis=mybir.AxisListType.X, op=ALU.add)
        lg = bigp.tile([128, MC, 1], F32)
        nc.scalar.activation(out=lg[:], in_=ssum[:], func=AF.Ln)
        nc.vector.tensor_tensor(
            out=h2[:], in0=h2[:], in1=lg[:].to_broadcast([128, MC, C]),
            op=ALU.subtract)
        nc.sync.dma_start(
            out=outT[:, :], in_=h2[:].rearrange("p m c -> p (m c)"))


# ------------------------------------------------------------------ runtime
_PROGRAM_CACHE = {}


def run(cfg, inputs):
    in_maps, sched, SBpad = preprocess(cfg, **inputs)
    key = (cfg.N, cfg.E, sched, SBpad)
    if key not in _PROGRAM_CACHE:
        _PROGRAM_CACHE[key] = build_program(cfg, sched, SBpad)
    nc = _PROGRAM_CACHE[key]
    res = None
    for attempt in range(3):
        try:
            res = run_bass_kernel_spmd(nc, in_maps,
                                       core_ids=list(range(cfg.ncores)))
            break
        except Exception:
            if attempt == 2:
                raise
    outs = []
    for k in range(cfg.ncores):
        oT = res.results[k]["outT"]                       # [128, MC*C]
        o = oT.reshape(128, cfg.MC, cfg.C).transpose(1, 0, 2).reshape(
            cfg.NLOC, cfg.C)
        outs.append(o)
    full = np.concatenate(outs, axis=0)[:cfg.N]
    return np.ascontiguousarray(full.astype(np.float32))


def kernel(x, edge_index, edge_relation, W1, b1, W2, b2):
    return run(CFG, dict(x=x, edge_index=edge_index,
                         edge_relation=edge_relation,
                         W1=W1, b1=b1, W2=W2, b2=b2))
